# revision 1
# baseline (speedup 1.0000x reference)
"""Trainium2 Bass kernel for nn_NodeGenerator (GNN message passing).

Strategy (8 NeuronCores, SPMD, no collectives):
  - Only candidate nodes (softmax class-0 > 0.5 and deg > 0) produce
    nonzero output rows.  Candidates are packed densely per core
    (~1.5K/core instead of 12.5K), so the MLP, activations and output
    DMA all run on ~12% of the nodes.
  - The neighbor-feature rows for each kept directed edge are packed on
    the host into a contiguous, degree-normalized f16 stream in
    (owner-window, tile, partition, lane) order, PACK edges of the same
    owner per PACK*64-element partition row.  The device streams it
    with large per-window DMAs (no per-row gather descriptors); ul/ctx0
    and the weights load on the GpSimd SWDGE ring in parallel.
  - Per owner window of 128 candidates: a one-hot matrix S built from
    iota/is_equal (DVE) and chained PE matmuls ps += S_t^T @ G_t give
    owner-major neighbor means [128, PACK*64] in fp32 PSUM; a strided
    DVE reduce folds the PACK lanes, a PE transpose (identity matmul)
    flips to feature-major, and the result lands in the ctx tile next
    to the candidates' own features.  The reduce/transpose for window w
    issue one window late so PE and DVE never stall on each other.
  - The 5-layer MLP runs feature-major over the packed candidate
    columns in chunks, each issued as soon as its ctx columns finish:
    f16 PE matmuls (fp32 PSUM) with fused fp32 bias/activation on ACT
    (DVE tensor_scalar for the tail chunks).  The prob head's P1 is
    host-folded with W3's feats block so it branches from h2 rather
    than the evicted g64, shortening the final dependency chain.  No
    masking needed - non-candidates never enter the device.
  - Per-core f16 outputs [67, COLS] + [1, COLS] are scattered on host
    into the zero-initialized full output.
"""

import numpy as np

N = 100000
D = 64
CORES = 8
NPC = N // CORES
PACK = 6      # edges of one owner packed per partition row
CHUNK = 512   # MLP column tile (psum free-dim limit for f32)


def _host_prep(node_features, node_operations, edge_index):
    f16 = np.float16
    X = np.asarray(node_features, np.float32)
    ops = np.asarray(node_operations, np.float64)
    ei = np.asarray(edge_index, np.int64)
    src, dst = ei[0], ei[1]
    U = np.concatenate([src, dst])
    V = np.concatenate([dst, src])
    deg = np.bincount(U, minlength=N)
    e = np.exp(ops - ops.max(axis=1, keepdims=True))
    p0 = e[:, 0] / e.sum(axis=1)
    mask = (p0 > 0.5) & (deg > 0)
    cand = np.where(mask)[0]
    if len(cand) == 0:
        return None
    ccore = cand // NPC
    ncand = np.bincount(ccore, minlength=CORES)
    NWIN = max(1, -(-int(ncand.max()) // 128))
    COLS = NWIN * 128

    ownerpos = np.full(N, -1, np.int64)
    cum = np.zeros(CORES + 1, np.int64)
    np.cumsum(ncand, out=cum[1:])
    ownerpos[cand] = np.arange(len(cand)) - cum[ccore]

    keep = mask[U]
    Uk, Vk = U[keep], V[keep]
    core = Uk // NPC
    oj = ownerpos[Uk]
    key = core * COLS + oj
    order = np.argsort(key, kind="stable")
    Uks, Vks = Uk[order], Vk[order]
    cores_s, ojs, keys = core[order], oj[order], key[order]

    counts = np.bincount(keys, minlength=CORES * COLS)
    starts = np.zeros(CORES * COLS + 1, np.int64)
    np.cumsum(counts, out=starts[1:])
    within = np.arange(len(keys)) - starts[keys]
    prow_in_owner = within // PACK
    lane = within % PACK

    q_u = -(-counts // PACK)                 # packed rows per owner slot
    qr = q_u.reshape(CORES, NWIN, 128)
    R = qr.sum(axis=2)                       # rows per (core, window)
    Rmax = np.maximum(R.max(axis=0), 1)      # valid rows per window
    TTW = -(-Rmax // 128)
    TTbase = np.zeros(NWIN + 1, np.int64)
    np.cumsum(TTW, out=TTbase[1:])
    SUMT = int(TTW.sum())
    TTmax = int(TTW.max())

    rowbase = np.zeros_like(qr)
    np.cumsum(qr[:, :, :-1], axis=2, out=rowbase[:, :, 1:])

    w_s = ojs >> 7
    slot_s = ojs & 127
    rw = rowbase[cores_s, w_s, slot_s] + prow_in_owner
    p_s = rw & 127
    t_s = rw >> 7
    gt = TTbase[w_s] + t_s

    rec = (1.0 / np.maximum(deg, 1)).astype(np.float32)
    scale = rec[Uks]

    ul = np.full((CORES, 128, SUMT), -1.0, f16)
    ul[cores_s, p_s, gt] = slot_s.astype(f16)
    G = np.zeros((CORES, 128, SUMT, PACK * D), f16)
    G.reshape(CORES, 128, SUMT, PACK, D)[cores_s, p_s, gt, lane] = \
        (X[Vks] * scale[:, None]).astype(f16)

    ctx0 = np.zeros((CORES, COLS, D), f16)
    for c in range(CORES):
        cc = cand[ccore == c]
        ctx0[c, :len(cc)] = X[cc].astype(f16)
    ctx0 = np.ascontiguousarray(ctx0.transpose(0, 2, 1))

    return dict(G=G, ul=ul, ctx0=ctx0, NWIN=NWIN, COLS=COLS, SUMT=SUMT,
                TTW=TTW.astype(np.int64), TTbase=TTbase, TTmax=TTmax,
                Rmax=Rmax.astype(np.int64), cand=cand, ccore=ccore,
                ncand=ncand)


def _build(prep):
    from concourse import bacc, mybir, tile
    f32 = mybir.dt.float32
    f16 = mybir.dt.float16
    AF = mybir.ActivationFunctionType
    ALU = mybir.AluOpType

    NWIN, COLS, SUMT = prep["NWIN"], prep["COLS"], prep["SUMT"]
    TTW, TTbase, TTmax = prep["TTW"], prep["TTbase"], prep["TTmax"]
    Rmax = prep["Rmax"]
    # MLP column chunks: full-width early; one medium final chunk so the
    # tail is a single serial chain (two interleaved chains head-of-line
    # block each other on the in-order engines)
    chunk_sz = []
    rem = COLS
    while rem > CHUNK:
        chunk_sz.append(CHUNK)
        rem -= CHUNK
    chunk_sz.append(rem)
    if chunk_sz[-1] < 256 and len(chunk_sz) >= 2:
        take = min(256 - chunk_sz[-1], chunk_sz[-2] - 128)
        chunk_sz[-2] -= take
        chunk_sz[-1] += take
    chunks = []
    off = 0
    for cs in chunk_sz:
        chunks.append((off, cs))
        off += cs

    nc = bacc.Bacc("TRN2", debug=False)

    def din(name, shape, dt=f32):
        return nc.dram_tensor(name, shape, dt, kind="ExternalInput")

    gh = din("g", [128, SUMT, PACK * D], f16)
    ulh = din("ul", [128, SUMT], f16)
    ctx0h = din("ctx0", [D, COLS], f16)
    w1h = din("w1", [2 * D, 128], f16)
    w2h = din("w2", [128, D], f16)
    w3h = din("w3", [D, 67], f16)
    p1h = din("p1", [D, 32], f16)
    p2h = din("p2", [32, 1], f16)
    b1h = din("b1", [128, 1])
    b2h = din("b2", [D, 1])
    b3h = din("b3", [67, 1])
    pb1h = din("pb1", [32, 1])
    pb2h = din("pb2", [1, 1])
    o67h = nc.dram_tensor("o67", [67, COLS], f16, kind="ExternalOutput")
    oph = nc.dram_tensor("op", [1, COLS], f16, kind="ExternalOutput")

    with tile.TileContext(nc) as tc:
        with (
            tc.tile_pool(name="const", bufs=1) as cpool,
            tc.tile_pool(name="gbuf", bufs=8) as gpool,
            tc.tile_pool(name="sbuf", bufs=3) as spool,
            tc.tile_pool(name="nbuf", bufs=2) as npool,
            tc.tile_pool(name="mlp", bufs=3) as mpool,
            tc.tile_pool(name="psw", bufs=2, space="PSUM") as psw,
            tc.tile_pool(name="pst", bufs=2, space="PSUM") as pst,
            tc.tile_pool(name="psb", bufs=2, space="PSUM") as psb,
            tc.tile_pool(name="pss", bufs=2, space="PSUM") as pss,
        ):
            # The SP ring carries only the G stream (plus output stores at
            # the end) so it flows from the first post-preamble cycle;
            # ul/ctx0/weights load on the GpSimd SWDGE ring in parallel.
            gtiles = []
            for w in range(NWIN):
                gb = int(TTbase[w])
                gt = int(TTW[w])
                g = gpool.tile([128, TTmax, PACK * D], f16, tag="g")
                nc.sync.dma_start(g[:, :gt, :], gh[:, gb:gb + gt, :])
                gtiles.append(g)

            iota = cpool.tile([128, 1, 128], f16)
            nc.gpsimd.iota(iota[:], pattern=[[0, 1], [1, 128]], base=0,
                           channel_multiplier=0,
                           allow_small_or_imprecise_dtypes=True)
            iop = cpool.tile([128, 1], f16)
            nc.gpsimd.iota(iop[:], pattern=[[0, 1]], base=0,
                           channel_multiplier=1,
                           allow_small_or_imprecise_dtypes=True)
            ident = cpool.tile([128, 128], f32)
            nc.vector.tensor_tensor(out=ident[:], in0=iota[:, 0, :],
                                    in1=iop[:].broadcast_to([128, 128]),
                                    op=ALU.is_equal)

            def load_const(h, shape, dt=f32):
                nm = f"c_{h.name}"
                t = cpool.tile(shape, dt, name=nm, tag=nm)
                nc.gpsimd.dma_start(t[:], h[:])
                return t

            ul_t = load_const(ulh, [128, SUMT], f16)
            ctx = cpool.tile([128, COLS], f16, name="ctx", tag="ctx")
            nc.gpsimd.dma_start(ctx[:D, :], ctx0h[:])
            w1_t = load_const(w1h, [2 * D, 128], f16)
            w2_t = load_const(w2h, [128, D], f16)
            w3_t = load_const(w3h, [D, 67], f16)
            p1_t = load_const(p1h, [D, 32], f16)
            p2_t = load_const(p2h, [32, 1], f16)
            b1_t = load_const(b1h, [128, 1])
            b2_t = load_const(b2h, [D, 1])
            b3_t = load_const(b3h, [67, 1])
            pb1_t = load_const(pb1h, [32, 1])
            pb2_t = load_const(pb2h, [1, 1])

            def build_S(w):
                tb, tw = int(TTbase[w]), int(TTW[w])
                S = spool.tile([128, TTmax, 128], f16, tag="S")
                nc.vector.tensor_tensor(
                    out=S[:, :tw, :],
                    in0=iota[:].broadcast_to([128, tw, 128]),
                    in1=ul_t[:, tb:tb + tw].broadcast_to([128, tw, 128]),
                    op=ALU.is_equal)
                return S

            def win_matmuls(w, S):
                tw = int(TTW[w])
                g = gtiles[w]
                v = int(Rmax[w]) & 127  # valid rows of the last tile
                ps = psw.tile([128, PACK * D], f32, tag="ps")
                for t in range(tw):
                    n = v if (v and t == tw - 1) else 128
                    nc.tensor.matmul(ps[:], lhsT=S[:n, t, :],
                                     rhs=g[:n, t, :],
                                     start=(t == 0), stop=(t == tw - 1))
                return ps

            def win_reduce(w, ps):
                nm = npool.tile([128, D], f32, tag="nm")
                nc.vector.tensor_reduce(
                    nm[:], ps[:].rearrange("p (q f) -> p f q", q=PACK),
                    axis=mybir.AxisListType.X, op=ALU.add)
                return nm

            def win_finish(w, nm):
                pt = pst.tile([D, 128], f32, tag="pt")
                nc.tensor.transpose(pt[:], nm[:], ident[:])
                nc.scalar.copy(ctx[D:, w * 128:(w + 1) * 128], pt[:])

            def mlp_chunk(base, cs, late):
                h1p = psb.tile([128, cs], f32, tag="big")
                nc.tensor.matmul(h1p[:], lhsT=w1_t[:],
                                 rhs=ctx[:, base:base + cs],
                                 start=True, stop=True)
                h1 = mpool.tile([128, cs], f16, tag="h1")
                if late:
                    nc.vector.tensor_scalar(out=h1[:], in0=h1p[:],
                                            scalar1=b1_t[:], scalar2=0.0,
                                            op0=ALU.add, op1=ALU.max)
                else:
                    nc.scalar.activation(out=h1[:], in_=h1p[:], func=AF.Relu,
                                         bias=b1_t[:], scale=1.0)

                h2p = psb.tile([D, cs], f32, tag="big")
                nc.tensor.matmul(h2p[:], lhsT=w2_t[:], rhs=h1[:],
                                 start=True, stop=True)
                h2 = mpool.tile([D, cs], f16, tag="h2")
                if late:
                    nc.vector.tensor_scalar(out=h2[:], in0=h2p[:],
                                            scalar1=b2_t[:], scalar2=0.0,
                                            op0=ALU.add, op1=ALU.max)
                else:
                    nc.scalar.activation(out=h2[:], in_=h2p[:], func=AF.Relu,
                                         bias=b2_t[:], scale=1.0)

                gp = pss.tile([67, cs], f32, tag="sm")
                nc.tensor.matmul(gp[:], lhsT=w3_t[:], rhs=h2[:],
                                 start=True, stop=True)
                o67 = mpool.tile([67, cs], f16, tag="o67")
                nc.scalar.activation(out=o67[:], in_=gp[:],
                                     func=AF.Identity, bias=b3_t[:],
                                     scale=1.0)
                nc.sync.dma_start(o67h[:, base:base + cs], o67[:])

                pp = pss.tile([32, cs], f32, tag="sm")
                nc.tensor.matmul(pp[:], lhsT=p1_t[:], rhs=h2[:],
                                 start=True, stop=True)
                pa = mpool.tile([32, cs], f16, tag="pa")
                nc.scalar.activation(out=pa[:], in_=pp[:], func=AF.Relu,
                                     bias=pb1_t[:], scale=1.0)

                prp = pss.tile([1, cs], f32, tag="sm")
                nc.tensor.matmul(prp[:], lhsT=p2_t[:], rhs=pa[:],
                                 start=True, stop=True)
                pr = mpool.tile([1, cs], f16, tag="pr")
                nc.scalar.activation(out=pr[:], in_=prp[:], func=AF.Sigmoid,
                                     bias=pb2_t[:], scale=1.0)
                nc.sync.dma_start(oph[:, base:base + cs], pr[:])

            # ---- Software-pipelined schedule: window w's reduce/transpose
            # issue one window later so PE/DVE never stall on each other;
            # MLP chunks issue as soon as their ctx columns are finished.
            lastwin = [(base + cs - 1) // 128 for base, cs in chunks]
            Sq = [build_S(w) for w in range(min(2, NWIN))]
            pending = None       # (w, ps)
            finished = -1        # highest window whose ctx copy is issued
            next_chunk = 0

            def flush_chunks():
                nonlocal next_chunk
                while (next_chunk < len(chunks)
                       and lastwin[next_chunk] <= finished):
                    base, cs = chunks[next_chunk]
                    mlp_chunk(base, cs, late=next_chunk == len(chunks) - 1)
                    next_chunk += 1

            for w in range(NWIN):
                ps = win_matmuls(w, Sq.pop(0))
                if w + 2 < NWIN:
                    Sq.append(build_S(w + 2))
                if pending is not None:
                    pw, pps = pending
                    win_finish(pw, win_reduce(pw, pps))
                    finished = pw
                    flush_chunks()
                pending = (w, ps)
            pw, pps = pending
            win_finish(pw, win_reduce(pw, pps))
            finished = pw
            flush_chunks()

    nc.compile()
    return nc


def _in_maps(prep, W1, b1, W2, b2, W3, b3, P1, pb1, P2, pb2):
    f16 = np.float16
    W3 = np.asarray(W3, np.float32)
    b3 = np.asarray(b3, np.float32)
    P1 = np.asarray(P1, np.float32)
    pb1 = np.asarray(pb1, np.float32)
    w3p = np.ascontiguousarray(np.concatenate([W3[:, 3:], W3[:, :3]], axis=1))
    b3p = np.concatenate([b3[3:], b3[:3]])
    # Fold the feats block of W3 into P1 so the prob head branches from h2
    # instead of the evicted g64: P1^T(W3g^T h2 + b3g) + pb1
    #   = (W3g P1)^T h2 + (P1^T b3g + pb1)
    w3g, b3g = w3p[:, :D], b3p[:D]
    p1eff = np.ascontiguousarray(w3g @ P1)
    pb1eff = P1.T @ b3g + pb1
    shared = {
        "w1": np.asarray(W1, np.float32).astype(f16),
        "w2": np.asarray(W2, np.float32).astype(f16),
        "w3": w3p.astype(f16),
        "p1": p1eff.astype(f16),
        "p2": np.asarray(P2, np.float32).astype(f16),
        "b1": np.asarray(b1, np.float32).reshape(-1, 1),
        "b2": np.asarray(b2, np.float32).reshape(-1, 1),
        "b3": b3p.astype(np.float32).reshape(-1, 1),
        "pb1": pb1eff.astype(np.float32).reshape(-1, 1),
        "pb2": np.asarray(pb2, np.float32).reshape(-1, 1),
    }
    maps = []
    for c in range(CORES):
        m = dict(shared)
        m["g"] = prep["G"][c]
        m["ul"] = prep["ul"][c]
        m["ctx0"] = prep["ctx0"][c]
        maps.append(m)
    return maps


def _assemble(prep, results):
    out = np.zeros((N, D + 4), np.float32)
    cand, ccore = prep["cand"], prep["ccore"]
    for c, r in enumerate(results):
        cc = cand[ccore == c]
        n_c = len(cc)
        o67 = r["o67"][:, :n_c].astype(np.float32)
        out[cc, 0:3] = o67[D:D + 3].T
        out[cc, 3:3 + D] = o67[:D].T
        out[cc, 3 + D] = r["op"][0, :n_c].astype(np.float32)
    return out


def kernel(**inputs):
    from concourse.bass_utils import run_bass_kernel_spmd
    prep = _host_prep(inputs["node_features"], inputs["node_operations"],
                      inputs["edge_index"])
    if prep is None:
        return np.zeros((N, D + 4), np.float32)
    nc = _build(prep)
    maps = _in_maps(prep, inputs["W1"], inputs["b1"], inputs["W2"],
                    inputs["b2"], inputs["W3"], inputs["b3"], inputs["P1"],
                    inputs["pb1"], inputs["P2"], inputs["pb2"])
    res = run_bass_kernel_spmd(nc, maps, core_ids=list(range(CORES)))
    return _assemble(prep, res.results)



# revision 4
# speedup vs baseline: 1.1972x; 1.1972x over previous
"""Trainium2 Bass kernel for nn_NodeGenerator (GNN message passing).

Strategy (8 NeuronCores, SPMD, no collectives):
  - Only candidate nodes (softmax class-0 > 0.5 and deg > 0) produce
    nonzero output rows.  Candidates are dealt to cores in 8 balanced
    contiguous chunks (<=1530/core -> 12 windows of 128, not 13), and
    within a core a greedy LPT pass balances neighbor-row counts across
    windows so every window packs to the same uniform TTW tile count.
  - The neighbor-feature rows for each kept directed edge are packed on
    the host into a degree-normalized fp8(e4m3) stream, PACK edges of
    the same owner per PACK*64-lane partition row.  The one-hot
    scatter matrix S is ALSO built on the host (fp8, exact 0/1) and
    shipped fused with G as one [128, TTW, 512] tile per window
    (cols 0:384 = G, 384:512 = S): a single DMA per window, no on-device
    iota/is_equal, no gpsimd ring, no ul load on the critical path.
  - All DMA triggers (weights blob, biases blob, ctx0, 12 windows) are
    issued up-front on the sync ring; windows land progressively and
    the PE consumes them at ~1us/window: chained fp8 matmuls
    ps += S_t^T @ G_t give owner-major neighbor means [128, 384] in
    fp32 PSUM; a strided DVE reduce folds the PACK lanes to f16, a PE
    transpose (f16 identity matmul) flips to feature-major, and ACT
    copies it into the ctx tile next to the candidates' own features.
    Window w's reduce/transpose issue one window late so PE and DVE
    never stall on each other.
  - The 5-layer MLP runs feature-major over the packed candidate
    columns in chunks issued as soon as their ctx columns finish:
    f16 PE matmuls (fp32 PSUM) with fused fp32 bias/activation on ACT
    (DVE tensor_scalar for the last chunk).  The prob head's P1 is
    host-folded with W3's feats block so it branches from h2.  Outputs
    merge into one [68, COLS] f16 tensor (row 67 = probability).
  - Per-core f16 outputs are scattered on host into the zero-initialized
    full output.
"""

import heapq

import numpy as np
import ml_dtypes

N = 100000
D = 64
CORES = 8
PACK = 6      # edges of one owner packed per partition row
CHUNK = 512   # MLP column tile (psum free-dim limit for f32)
F8 = ml_dtypes.float8_e4m3fn


def _host_prep(node_features, node_operations, edge_index):
    X = np.asarray(node_features, np.float32)
    ops = np.asarray(node_operations, np.float64)
    ei = np.asarray(edge_index, np.int64)
    src, dst = ei[0], ei[1]
    U = np.concatenate([src, dst])
    V = np.concatenate([dst, src])
    deg = np.bincount(U, minlength=N)
    e = np.exp(ops - ops.max(axis=1, keepdims=True))
    p0 = e[:, 0] / e.sum(axis=1)
    mask = (p0 > 0.5) & (deg > 0)
    cand = np.where(mask)[0]
    NC = len(cand)
    if NC == 0:
        return None

    # balanced contiguous split of candidates across cores
    sizes = [len(a) for a in np.array_split(cand, CORES)]
    maxc = max(sizes)
    NWIN = max(1, -(-maxc // 128))
    COLS = NWIN * 128
    ccore = np.repeat(np.arange(CORES), sizes)
    cum = np.zeros(CORES + 1, np.int64)
    np.cumsum(sizes, out=cum[1:])
    lpos = np.arange(NC) - cum[ccore]          # local candidate index

    q = -(-deg[cand] // PACK)                  # packed rows per candidate

    # greedy LPT: balance rows across NWIN windows per core (<=128 owners)
    win = np.empty(NC, np.int64)
    slot = np.empty(NC, np.int64)
    rowbase = np.empty(NC, np.int64)
    maxrows = 0
    for c in range(CORES):
        i0, i1 = cum[c], cum[c + 1]
        qi = q[i0:i1]
        order = np.argsort(-qi, kind="stable")
        heap = [(0, w) for w in range(NWIN)]
        heapq.heapify(heap)
        count = np.zeros(NWIN, np.int64)
        rows = np.zeros(NWIN, np.int64)
        wloc = np.empty(len(qi), np.int64)
        for j in order:
            while True:
                r, w = heapq.heappop(heap)
                if count[w] < 128:
                    break
            wloc[j] = w
            count[w] += 1
            rows[w] += qi[j]
            heapq.heappush(heap, (int(rows[w]), w))
        maxrows = max(maxrows, int(rows.max()))
        # slots + row offsets in window, in local-index order
        sl = np.zeros(len(qi), np.int64)
        rb = np.zeros(len(qi), np.int64)
        scount = np.zeros(NWIN, np.int64)
        srows = np.zeros(NWIN, np.int64)
        for j in range(len(qi)):
            w = wloc[j]
            sl[j] = scount[w]
            rb[j] = srows[w]
            scount[w] += 1
            srows[w] += qi[j]
        win[i0:i1] = wloc
        slot[i0:i1] = sl
        rowbase[i0:i1] = rb
    TTW = max(1, -(-maxrows // 128))
    SUMT = NWIN * TTW

    # per-candidate lookup tables over node ids
    cwin = np.full(N, -1, np.int64)
    cslot = np.full(N, -1, np.int64)
    crow = np.full(N, -1, np.int64)
    ccore_n = np.full(N, -1, np.int64)
    cwin[cand] = win
    cslot[cand] = slot
    crow[cand] = rowbase
    ccore_n[cand] = ccore

    # kept directed edges, grouped per owner
    keep = mask[U]
    Uk, Vk = U[keep], V[keep]
    key = cwin[Uk] * 128 + cslot[Uk] + (ccore_n[Uk] * COLS)
    order = np.argsort(key, kind="stable")
    Uks, Vks = Uk[order], Vk[order]
    keys = key[order]
    counts = np.bincount(keys, minlength=CORES * COLS)
    starts = np.zeros(CORES * COLS + 1, np.int64)
    np.cumsum(counts, out=starts[1:])
    within = np.arange(len(keys)) - starts[keys]

    prow = crow[Uks] + within // PACK          # row within the window
    lane = within % PACK
    part = prow & 127
    gt = cwin[Uks] * TTW + (prow >> 7)         # global tile index
    core_e = ccore_n[Uks]

    rec = (1.0 / np.maximum(deg, 1)).astype(np.float32)
    vals = (X[Vks] * rec[Uks][:, None]).astype(F8)

    WS = np.zeros((CORES, 128, SUMT, 512), F8)
    WS.reshape(CORES, 128, SUMT, 8, 64)[core_e, part, gt, lane] = vals

    # one-hot S columns (384 + slot), one per packed row
    rcore = np.repeat(ccore, q)
    rwin = np.repeat(win, q)
    rslot = np.repeat(slot, q)
    qs = np.zeros(NC + 1, np.int64)
    np.cumsum(q, out=qs[1:])
    rin = np.arange(qs[-1]) - np.repeat(qs[:-1], q)   # row index within owner
    rrow = np.repeat(rowbase, q) + rin
    WS[rcore, rrow & 127, rwin * TTW + (rrow >> 7), 384 + rslot] = np.asarray(1.0, F8)

    # candidates' own features, feature-major per core
    col = win * 128 + slot
    ctx0 = np.zeros((CORES, COLS, D), np.float16)
    ctx0[ccore, col] = X[cand].astype(np.float16)
    ctx0 = np.ascontiguousarray(ctx0.transpose(0, 2, 1))

    return dict(WS=WS, ctx0=ctx0, NWIN=NWIN, COLS=COLS, SUMT=SUMT, TTW=TTW,
                cand=cand, ccore=ccore, col=col)


def _chunks(COLS):
    sz = []
    rem = COLS
    while rem > CHUNK:
        sz.append(CHUNK)
        rem -= CHUNK
    sz.append(rem)
    if sz[-1] < 256 and len(sz) >= 2:
        take = min(256 - sz[-1], sz[-2] - 128)
        sz[-2] -= take
        sz[-1] += take
    if sz[-1] == 512:  # shorten the serial tail chain
        sz[-1] = 256
        sz.append(256)
    out = []
    off = 0
    for cs in sz:
        out.append((off, cs))
        off += cs
    return out


def _build(prep):
    from concourse import bacc, mybir, tile
    f32 = mybir.dt.float32
    f16 = mybir.dt.float16
    f8 = mybir.dt.float8e4
    AF = mybir.ActivationFunctionType
    ALU = mybir.AluOpType

    NWIN, COLS, SUMT, TTW = (prep["NWIN"], prep["COLS"], prep["SUMT"],
                             prep["TTW"])
    chunks = _chunks(COLS)

    nc = bacc.Bacc("TRN2", debug=False)

    wsh = nc.dram_tensor("ws", [128, SUMT, 512], f8, kind="ExternalInput")
    ctx0h = nc.dram_tensor("ctx0", [D, COLS], f16, kind="ExternalInput")
    wfh = nc.dram_tensor("wf", [128, 420], f16, kind="ExternalInput")
    bfh = nc.dram_tensor("bf", [128, 5], f32, kind="ExternalInput")
    o67h = nc.dram_tensor("o67", [67, COLS], f16, kind="ExternalOutput")
    oph = nc.dram_tensor("op", [1, COLS], f16, kind="ExternalOutput")

    with tile.TileContext(nc) as tc:
        with (
            tc.tile_pool(name="const", bufs=1) as cpool,
            tc.tile_pool(name="gbuf", bufs=NWIN) as gpool,
            tc.tile_pool(name="nbuf", bufs=2) as npool,
            tc.tile_pool(name="mlp", bufs=3) as mpool,
            tc.tile_pool(name="psw", bufs=3, space="PSUM") as psw,
            tc.tile_pool(name="pst", bufs=1, space="PSUM") as pst,
            tc.tile_pool(name="psb", bufs=2, space="PSUM") as psb,
            tc.tile_pool(name="pss", bufs=2, space="PSUM") as pss,
        ):
            # all input DMAs issue up-front on the sync ring: constants
            # first (small, needed by the MLP), then the window stream.
            wt = cpool.tile([128, 420], f16, name="wt", tag="wt")
            nc.sync.dma_start(wt[:], wfh[:])
            bt = cpool.tile([128, 5], f32, name="bt", tag="bt")
            nc.sync.dma_start(bt[:], bfh[:])
            ctx = cpool.tile([128, COLS], f16, name="ctx", tag="ctx")
            nc.sync.dma_start(ctx[:D, :], ctx0h[:])
            wins = []
            for w in range(NWIN):
                g = gpool.tile([128, TTW, 512], f8, tag="g")
                nc.sync.dma_start(g[:], wsh[:, w * TTW:(w + 1) * TTW, :])
                wins.append(g)

            ident = wt[:, 0:128]
            w1_t = wt[:, 128:256]
            w2_t = wt[:, 256:320]
            w3_t = wt[:64, 320:387]
            p1_t = wt[:64, 387:419]
            p2_t = wt[:32, 419:420]
            b1_t = bt[:, 0:1]
            b2_t = bt[:64, 1:2]
            b3_t = bt[:67, 2:3]
            pb1_t = bt[:32, 3:4]
            pb2_t = bt[:1, 4:5]

            def win_matmuls(w):
                g = wins[w]
                ps = psw.tile([128, 384], f32, tag="ps")
                for t in range(TTW):
                    nc.tensor.matmul(ps[:], lhsT=g[:, t, 384:512],
                                     rhs=g[:, t, 0:384],
                                     start=(t == 0), stop=(t == TTW - 1))
                return ps

            def win_finish(w, ps):
                nm = npool.tile([128, D], f16, tag="nm")
                with nc.allow_low_precision(reason="6-way fold to f16 ctx"):
                    nc.vector.tensor_reduce(
                        nm[:], ps[:].rearrange("p (q f) -> p f q", q=PACK),
                        axis=mybir.AxisListType.X, op=ALU.add)
                pt = pst.tile([D, 128], f16, tag="pt")
                nc.tensor.transpose(pt[:], nm[:], ident)
                nc.scalar.copy(ctx[D:, w * 128:(w + 1) * 128], pt[:])

            def mlp_chunk(base, cs, late):
                h1p = psb.tile([128, cs], f32, tag="big")
                nc.tensor.matmul(h1p[:], lhsT=w1_t,
                                 rhs=ctx[:, base:base + cs],
                                 start=True, stop=True)
                h1 = mpool.tile([128, cs], f16, tag="h1")
                if late:
                    nc.vector.tensor_scalar(out=h1[:], in0=h1p[:],
                                            scalar1=b1_t, scalar2=0.0,
                                            op0=ALU.add, op1=ALU.max)
                else:
                    nc.scalar.activation(out=h1[:], in_=h1p[:], func=AF.Relu,
                                         bias=b1_t, scale=1.0)

                h2p = psb.tile([D, cs], f32, tag="big")
                nc.tensor.matmul(h2p[:], lhsT=w2_t, rhs=h1[:],
                                 start=True, stop=True)
                h2 = mpool.tile([D, cs], f16, tag="h2")
                if late:
                    nc.vector.tensor_scalar(out=h2[:], in0=h2p[:],
                                            scalar1=b2_t, scalar2=0.0,
                                            op0=ALU.add, op1=ALU.max)
                else:
                    nc.scalar.activation(out=h2[:], in_=h2p[:], func=AF.Relu,
                                         bias=b2_t, scale=1.0)

                o67 = mpool.tile([67, cs], f16, tag="o67")
                gp = pss.tile([67, cs], f32, tag="sm")
                nc.tensor.matmul(gp[:], lhsT=w3_t, rhs=h2[:],
                                 start=True, stop=True)
                nc.scalar.activation(out=o67[:], in_=gp[:],
                                     func=AF.Identity, bias=b3_t, scale=1.0)
                nc.sync.dma_start(o67h[:, base:base + cs], o67[:])

                pp = pss.tile([32, cs], f32, tag="sm")
                nc.tensor.matmul(pp[:], lhsT=p1_t, rhs=h2[:],
                                 start=True, stop=True)
                pa = mpool.tile([32, cs], f16, tag="pa")
                nc.scalar.activation(out=pa[:], in_=pp[:], func=AF.Relu,
                                     bias=pb1_t, scale=1.0)
                prp = pss.tile([1, cs], f32, tag="sm")
                nc.tensor.matmul(prp[:], lhsT=p2_t, rhs=pa[:],
                                 start=True, stop=True)
                pr = mpool.tile([1, cs], f16, tag="pr")
                nc.scalar.activation(out=pr[:], in_=prp[:],
                                     func=AF.Sigmoid, bias=pb2_t, scale=1.0)
                nc.sync.dma_start(oph[:, base:base + cs], pr[:])

            # ---- Software-pipelined schedule: window w's reduce/transpose
            # issue one window later so PE/DVE never stall on each other;
            # MLP chunks issue as soon as their ctx columns are finished.
            lastwin = [(base + cs - 1) // 128 for base, cs in chunks]
            pending = None
            finished = -1
            next_chunk = 0

            def flush_chunks():
                nonlocal next_chunk
                while (next_chunk < len(chunks)
                       and lastwin[next_chunk] <= finished):
                    base, cs = chunks[next_chunk]
                    mlp_chunk(base, cs, late=next_chunk == len(chunks) - 1)
                    next_chunk += 1

            for w in range(NWIN):
                ps = win_matmuls(w)
                if pending is not None:
                    pw, pps = pending
                    win_finish(pw, pps)
                    finished = pw
                    flush_chunks()
                pending = (w, ps)
            pw, pps = pending
            win_finish(pw, pps)
            finished = pw
            flush_chunks()

    nc.compile()
    return nc


def _in_maps(prep, W1, b1, W2, b2, W3, b3, P1, pb1, P2, pb2):
    f16 = np.float16
    W1 = np.asarray(W1, np.float32)
    W2 = np.asarray(W2, np.float32)
    W3 = np.asarray(W3, np.float32)
    b3 = np.asarray(b3, np.float32)
    P1 = np.asarray(P1, np.float32)
    pb1 = np.asarray(pb1, np.float32)
    P2 = np.asarray(P2, np.float32)
    w3p = np.ascontiguousarray(np.concatenate([W3[:, 3:], W3[:, :3]], axis=1))
    b3p = np.concatenate([b3[3:], b3[:3]])
    # Fold the feats block of W3 into P1 so the prob head branches from h2
    # instead of the evicted g64: P1^T(W3g^T h2 + b3g) + pb1
    #   = (W3g P1)^T h2 + (P1^T b3g + pb1)
    w3g, b3g = w3p[:, :D], b3p[:D]
    p1eff = w3g @ P1
    pb1eff = P1.T @ b3g + pb1

    wf = np.zeros((128, 420), f16)
    wf[:, 0:128] = np.eye(128, dtype=f16)
    wf[:, 128:256] = W1.astype(f16)
    wf[:, 256:320] = W2.astype(f16)
    wf[:64, 320:387] = w3p.astype(f16)
    wf[:64, 387:419] = p1eff.astype(f16)
    wf[:32, 419:420] = P2.astype(f16)
    bf = np.zeros((128, 5), np.float32)
    bf[:, 0] = np.asarray(b1, np.float32)
    bf[:64, 1] = np.asarray(b2, np.float32)
    bf[:67, 2] = b3p
    bf[:32, 3] = pb1eff
    bf[0, 4] = np.asarray(pb2, np.float32).ravel()[0]

    maps = []
    for c in range(CORES):
        maps.append({
            "ws": prep["WS"][c],
            "ctx0": prep["ctx0"][c],
            "wf": wf,
            "bf": bf,
        })
    return maps


def _assemble(prep, results):
    out = np.zeros((N, D + 4), np.float32)
    cand, ccore, col = prep["cand"], prep["ccore"], prep["col"]
    for c, r in enumerate(results):
        sel = ccore == c
        ids = cand[sel]
        cc = col[sel]
        o67 = r["o67"][:, cc].astype(np.float32)
        out[ids, 0:3] = o67[D:D + 3].T
        out[ids, 3:3 + D] = o67[:D].T
        out[ids, 3 + D] = r["op"][0, cc].astype(np.float32)
    return out


def kernel(**inputs):
    from concourse.bass_utils import run_bass_kernel_spmd
    prep = _host_prep(inputs["node_features"], inputs["node_operations"],
                      inputs["edge_index"])
    if prep is None:
        return np.zeros((N, D + 4), np.float32)
    nc = _build(prep)
    maps = _in_maps(prep, inputs["W1"], inputs["b1"], inputs["W2"],
                    inputs["b2"], inputs["W3"], inputs["b3"], inputs["P1"],
                    inputs["pb1"], inputs["P2"], inputs["pb2"])
    res = run_bass_kernel_spmd(nc, maps, core_ids=list(range(CORES)))
    return _assemble(prep, res.results)
